# revision 19
# baseline (speedup 1.0000x reference)
"""Trainium2 Bass kernel for nn_MAB (dense transformer block).

Reference computation (B=32, N=512, D=512, H=8, dh=64):
    q = (Q @ Wq.T + bq)  k = (K @ Wk.T + bk)  v = (K @ Wv.T + bv)
    scores = einsum("bqhd,bkhd->bhqk", q, k) / sqrt(512)
    A = softmax(scores, axis=2)            # over the QUERY axis!
    attn = einsum("bhqk,bkhd->bqhd", A, v).reshape(B, N, D)
    out = Q + attn @ Wo.T + bo
    ffn = relu(out @ W1.T + b1) @ W2.T + b2
    return out + ffn

Strategy: pure data-parallel over batch: 8 cores x 4 batches, zero
collectives.  On-chip activations are kept in TRANSPOSED layout
([feature, token], feature on partitions) so every matmul contracts over
partitions without any on-chip transposes; host pre-transposes Q/K and
the weights, and re-transposes the output.  Matmuls run in float32r
(full PE rate at moving-dim >= 256, ~tf32 precision).

Softmax over the query axis is computed on scores^T tiles ([k, q],
q on the free axis): ACT exp with fused free-axis accumulation, then the
reciprocal row-sums are folded into v (64x fewer elements than A).
Attention runs per head-PAIR: the two heads of a pair occupy disjoint
row groups (scores, K=64) / col groups (attn-apply, M=64) of the PE
array via tile_position, so their matmuls execute concurrently.
E and v~ are bf16 (attn matmul at full rate; softmax tolerance is wide).
"""

import math
import os
import sys

import numpy as np

sys.path.insert(0, "/opt/trn_rl_repo")

import concourse.bass as bass  # noqa: E402
import concourse.tile as tile  # noqa: E402
from concourse import bacc  # noqa: E402
from concourse import mybir  # noqa: E402
from concourse.bass_utils import run_bass_kernel_spmd  # noqa: E402

F32 = mybir.dt.float32
F32R = mybir.dt.float32r
BF16 = mybir.dt.bfloat16
AF = mybir.ActivationFunctionType
ALU = mybir.AluOpType

B, N, D, H = 32, 512, 512, 8
DH = D // H  # 64
NCORES = 8
BLOC = B // NCORES  # 4 batches per core
SCALE = 1.0 / math.sqrt(512.0)
P = 128
KC = D // P  # 4 contraction chunks
MC = D // P  # 4 output-feature chunks

_CACHE = {}


def _build_program(with_bias):
    nc = bacc.Bacc("TRN2", target_bir_lowering=False, debug=False,
                   num_devices=NCORES)

    # DRAM I/O ------------------------------------------------------------
    qT_d = nc.dram_tensor("qT", [BLOC, D, N], F32R, kind="ExternalInput").ap()
    kT_d = nc.dram_tensor("kT", [BLOC, D, N], F32R, kind="ExternalInput").ap()
    w_d = {}
    for nm in ("wq", "wk", "wv", "wo", "w1", "w2"):
        w_d[nm] = nc.dram_tensor(nm, [D, D], F32R, kind="ExternalInput").ap()
    b_d = {}
    if with_bias:
        for nm in ("bq", "bk", "bv", "bo", "b1", "b2"):
            b_d[nm] = nc.dram_tensor(nm, [D], F32, kind="ExternalInput").ap()
    outT_d = nc.dram_tensor("outT", [BLOC, D, N], F32,
                            kind="ExternalOutput").ap()

    qT_v = qT_d.rearrange("b (o p) t -> b p o t", p=P)
    kT_v = kT_d.rearrange("b (o p) t -> b p o t", p=P)
    outT_v = outT_d.rearrange("b (o p) t -> b p o t", p=P)
    w_v = {k: v.rearrange("(o p) n -> p o n", p=P) for k, v in w_d.items()}
    b_v = {k: v.rearrange("(o p) -> p o", p=P) for k, v in b_d.items()}

    with tile.TileContext(nc) as tc:
        with (
            tc.tile_pool(name="weights", bufs=1) as wpool,
            tc.tile_pool(name="qin", bufs=2) as qin_pool,
            tc.tile_pool(name="kin", bufs=2) as kin_pool,
            tc.tile_pool(name="proj", bufs=2) as proj_pool,
            tc.tile_pool(name="exp", bufs=4) as exp_pool,
            tc.tile_pool(name="rsum", bufs=4) as rsum_pool,
            tc.tile_pool(name="attn", bufs=2) as attn_pool,
            tc.tile_pool(name="ffn", bufs=2) as ffn_pool,
            tc.tile_pool(name="h1p", bufs=2) as h1_pool,
            tc.tile_pool(name="fin", bufs=2) as fin_pool,
            tc.tile_pool(name="psA", bufs=5, space="PSUM") as psA,
            tc.tile_pool(name="psS", bufs=2, space="PSUM") as psS,
            tc.tile_pool(name="psB", bufs=1, space="PSUM") as psB,
        ):
            # ---- resident weights/biases --------------------------------
            w_sb = {}
            for nm in ("wq", "wk", "wv", "wo", "w1", "w2"):
                w_sb[nm] = wpool.tile([P, KC, D], F32R, tag=f"w_{nm}",
                                      name=f"w_{nm}")
            nc.sync.dma_start(out=w_sb["wq"][:], in_=w_v["wq"])
            qt0 = qin_pool.tile([P, KC, N], F32R, tag="qt", name="qt0")
            nc.sync.dma_start(out=qt0[:], in_=qT_v[0])
            nc.sync.dma_start(out=w_sb["wk"][:], in_=w_v["wk"])
            kt0 = kin_pool.tile([P, KC, N], F32R, tag="kt", name="kt0")
            nc.sync.dma_start(out=kt0[:], in_=kT_v[0])
            nc.sync.dma_start(out=w_sb["wv"][:], in_=w_v["wv"])
            b_sb = {}
            bv_bc = None
            if with_bias:
                for nm in ("bq", "bk", "bo", "b1", "b2"):
                    b_sb[nm] = wpool.tile([P, MC], F32, tag=f"b_{nm}",
                                          name=f"b_{nm}")
                    nc.sync.dma_start(out=b_sb[nm][:], in_=b_v[nm])
                bv_bc = wpool.tile([P, D], F32, tag="bv_bc")
                bv_src = bass.AP(tensor=b_d["bv"].tensor,
                                 offset=b_d["bv"].offset,
                                 ap=[[0, P], *b_d["bv"].ap])
                nc.sync.dma_start(out=bv_bc[:], in_=bv_src)

            def linearT(dst, rhs_src, wname, bias):
                """dst[:, m, :] ([P, MC, N] transposed layout) = W @ rhs + b"""
                for m in range(MC):
                    ps = psA.tile([P, N], F32, tag="psA")
                    for kc in range(KC):
                        nc.tensor.matmul(
                            ps, lhsT=w_sb[wname][:, kc, m * P:(m + 1) * P],
                            rhs=rhs_src[:, kc, :],
                            start=(kc == 0), stop=(kc == KC - 1))
                    if with_bias:
                        nc.vector.tensor_scalar(
                            out=dst[:, m, :], in0=ps,
                            scalar1=b_sb[bias][:, m:m + 1], scalar2=None,
                            op0=ALU.add)
                    else:
                        nc.vector.tensor_copy(out=dst[:, m, :], in_=ps)

            for b in range(BLOC):
                # ---- load inputs (b=0 pre-issued above) ----------------
                if b == 0:
                    qt_b, kt_b = qt0, kt0
                else:
                    qt_b = qin_pool.tile([P, KC, N], F32R, tag="qt")
                    nc.sync.dma_start(out=qt_b[:], in_=qT_v[b])
                    kt_b = kin_pool.tile([P, KC, N], F32R, tag="kt")
                    nc.sync.dma_start(out=kt_b[:], in_=kT_v[b])

                # ---- projections (transposed layout) -------------------
                qh = proj_pool.tile([P, MC, N], BF16, tag="qh")
                linearT(qh, qt_b, "wq", "bq")
                kh = proj_pool.tile([P, MC, N], BF16, tag="kh")
                linearT(kh, kt_b, "wk", "bk")

                # v in natural layout [tok, d']: 4 token chunks
                v_b = proj_pool.tile([P, KC, D], BF16, tag="v")
                for tt in range(KC):
                    ps = psA.tile([P, D], F32, tag="psA")
                    for kc in range(KC):
                        nc.tensor.matmul(
                            ps, lhsT=kt_b[:, kc, tt * P:(tt + 1) * P],
                            rhs=w_sb["wv"][:, kc, :],
                            start=(kc == 0), stop=(kc == KC - 1))
                    if with_bias:
                        nc.vector.tensor_tensor(
                            out=v_b[:, tt, :], in0=ps, in1=bv_bc[:],
                            op=ALU.add)
                    else:
                        nc.vector.tensor_copy(out=v_b[:, tt, :], in_=ps)

                if b == 0:
                    # deferred weight loads: DMA overlaps attention of b=0
                    for nm in ("wo", "w1", "w2"):
                        nc.sync.dma_start(out=w_sb[nm][:], in_=w_v[nm])

                # ---- attention, head pairs -----------------------------
                # pair hp = heads (2hp, 2hp+1): rows 0-63 / 64-127 of
                # feature chunk hp.  Scores row-packed (K=64 x2), attn
                # col-packed (M=64 x2) into one [128, N] psum.
                attnT = attn_pool.tile([P, MC, N], F32R, tag="attnT")
                for hp in range(MC):
                    e0 = exp_pool.tile([P, KC, N], BF16, tag="e", name="e0")
                    e1 = exp_pool.tile([P, KC, N], BF16, tag="e", name="e1")
                    racc = rsum_pool.tile([P, KC, 2], F32, tag="racc")
                    for j in range(KC):
                        js = slice(j * P, (j + 1) * P)
                        ps0 = psS.tile([P, N], F32, tag="psS")
                        nc.tensor.matmul(
                            ps0, lhsT=kh[0:DH, hp, js], rhs=qh[0:DH, hp, :],
                            start=True, stop=True)
                        ps1 = psS.tile([P, N], F32, tag="psS")
                        nc.tensor.matmul(
                            ps1, lhsT=kh[DH:P, hp, js], rhs=qh[DH:P, hp, :],
                            start=True, stop=True)
                        nc.scalar.activation(
                            out=e0[:, j, :], in_=ps0, func=AF.Exp,
                            scale=SCALE, accum_out=racc[:, j, 0:1])
                        nc.scalar.activation(
                            out=e1[:, j, :], in_=ps1, func=AF.Exp,
                            scale=SCALE, accum_out=racc[:, j, 1:2])
                    rrec = rsum_pool.tile([P, KC, 2], F32, tag="rrec")
                    nc.vector.reciprocal(out=rrec[:], in_=racc[:])
                    vt0 = rsum_pool.tile([P, KC, DH], BF16, tag="vt",
                                         name="vt0")
                    vt1 = rsum_pool.tile([P, KC, DH], BF16, tag="vt",
                                         name="vt1")
                    nc.vector.tensor_tensor(
                        out=vt0[:], in0=v_b[:, :, 2 * hp * DH:(2 * hp + 1) * DH],
                        in1=rrec[:, :, 0:1].to_broadcast((P, KC, DH)),
                        op=ALU.mult)
                    nc.vector.tensor_tensor(
                        out=vt1[:], in0=v_b[:, :, (2 * hp + 1) * DH:(2 * hp + 2) * DH],
                        in1=rrec[:, :, 1:2].to_broadcast((P, KC, DH)),
                        op=ALU.mult)
                    ps = psB.tile([P, N], F32, tag="psB")
                    for j in range(KC):
                        nc.tensor.matmul(
                            ps[0:DH, :], lhsT=vt0[:, j, :], rhs=e0[:, j, :],
                            start=(j == 0), stop=(j == KC - 1),
                            tile_position=(0, 0))
                        nc.tensor.matmul(
                            ps[DH:P, :], lhsT=vt1[:, j, :], rhs=e1[:, j, :],
                            start=(j == 0), stop=(j == KC - 1),
                            tile_position=(0, DH))
                    nc.vector.tensor_copy(out=attnT[:, hp, :], in_=ps)

                # ---- out = Q + attn @ Wo.T + bo (transposed) -----------
                outT_b = ffn_pool.tile([P, MC, N], F32R, tag="outT")
                for m in range(MC):
                    ps = psA.tile([P, N], F32, tag="psA")
                    for kc in range(KC):
                        nc.tensor.matmul(
                            ps, lhsT=w_sb["wo"][:, kc, m * P:(m + 1) * P],
                            rhs=attnT[:, kc, :],
                            start=(kc == 0), stop=(kc == KC - 1))
                    if with_bias:
                        nc.vector.tensor_scalar(
                            out=outT_b[:, m, :], in0=ps,
                            scalar1=b_sb["bo"][:, m:m + 1], scalar2=None,
                            op0=ALU.add)
                        nc.vector.tensor_tensor(
                            out=outT_b[:, m, :], in0=outT_b[:, m, :],
                            in1=qt_b[:, m, :], op=ALU.add)
                    else:
                        nc.vector.tensor_tensor(
                            out=outT_b[:, m, :], in0=ps,
                            in1=qt_b[:, m, :], op=ALU.add)

                # ---- ffn h1 = relu(W1 out^T + b1) ----------------------
                h1 = h1_pool.tile([P, MC, N], F32R, tag="h1")
                for m in range(MC):
                    ps = psA.tile([P, N], F32, tag="psA")
                    for kc in range(KC):
                        nc.tensor.matmul(
                            ps, lhsT=w_sb["w1"][:, kc, m * P:(m + 1) * P],
                            rhs=outT_b[:, kc, :],
                            start=(kc == 0), stop=(kc == KC - 1))
                    nc.vector.tensor_scalar(
                        out=h1[:, m, :], in0=ps,
                        scalar1=b_sb["b1"][:, m:m + 1] if with_bias else 0.0,
                        scalar2=0.0,
                        op0=ALU.add, op1=ALU.max)

                # ---- final = out + W2 h1 + b2, DMA out -----------------
                for m in range(MC):
                    ps = psA.tile([P, N], F32, tag="psA")
                    for kc in range(KC):
                        nc.tensor.matmul(
                            ps, lhsT=w_sb["w2"][:, kc, m * P:(m + 1) * P],
                            rhs=h1[:, kc, :],
                            start=(kc == 0), stop=(kc == KC - 1))
                    fin = fin_pool.tile([P, N], F32, tag="fin")
                    if with_bias:
                        nc.scalar.activation(
                            out=fin[:], in_=ps, func=AF.Identity,
                            bias=b_sb["b2"][:, m:m + 1], scale=1.0)
                        nc.vector.tensor_tensor(
                            out=fin[:], in0=fin[:], in1=outT_b[:, m, :],
                            op=ALU.add)
                    else:
                        nc.vector.tensor_tensor(
                            out=fin[:], in0=ps, in1=outT_b[:, m, :],
                            op=ALU.add)
                    nc.sync.dma_start(out=outT_v[b][:, m, :], in_=fin[:])

    nc.compile()
    return nc


def kernel(Q, K, Wq, bq, Wk, bk, Wv, bv, Wo, bo, W1, b1, W2, b2):
    Q = np.asarray(Q, dtype=np.float32)
    K = np.asarray(K, dtype=np.float32)

    biases = {nm: np.asarray(v, np.float32) for nm, v in
              (("bq", bq), ("bk", bk), ("bv", bv),
               ("bo", bo), ("b1", b1), ("b2", b2))}
    with_bias = any(np.any(v) for v in biases.values())

    key = ("nc", with_bias)
    if key not in _CACHE:
        _CACHE[key] = _build_program(with_bias)
    nc = _CACHE[key]

    common = {
        "wq": np.ascontiguousarray(np.asarray(Wq, np.float32).T),
        "wk": np.ascontiguousarray(np.asarray(Wk, np.float32).T),
        "wv": np.ascontiguousarray(np.asarray(Wv, np.float32).T),
        "wo": np.ascontiguousarray(np.asarray(Wo, np.float32).T),
        "w1": np.ascontiguousarray(np.asarray(W1, np.float32).T),
        "w2": np.ascontiguousarray(np.asarray(W2, np.float32).T),
    }
    if with_bias:
        common.update(biases)
    in_maps = []
    for c in range(NCORES):
        sl = slice(c * BLOC, (c + 1) * BLOC)
        in_maps.append({
            "qT": np.ascontiguousarray(Q[sl].transpose(0, 2, 1)),
            "kT": np.ascontiguousarray(K[sl].transpose(0, 2, 1)),
            **common,
        })

    trace = bool(int(os.environ.get("KERNEL_TRACE", "0")))
    res = run_bass_kernel_spmd(nc, in_maps, core_ids=list(range(NCORES)),
                               trace=trace)
    if trace and res.exec_time_ns is not None:
        print(f"HW exec time: {res.exec_time_ns} ns")
        if res.instructions_and_trace is not None:
            print("trace:", res.instructions_and_trace[1])

    out = np.empty((B, N, D), np.float32)
    for c in range(NCORES):
        out[c * BLOC:(c + 1) * BLOC] = res.results[c]["outT"].transpose(0, 2, 1)
    return out


# revision 20
# speedup vs baseline: 1.0417x; 1.0417x over previous
"""Trainium2 Bass kernel for nn_MAB (dense transformer block).

Reference computation (B=32, N=512, D=512, H=8, dh=64):
    q = (Q @ Wq.T + bq)  k = (K @ Wk.T + bk)  v = (K @ Wv.T + bv)
    scores = einsum("bqhd,bkhd->bhqk", q, k) / sqrt(512)
    A = softmax(scores, axis=2)            # over the QUERY axis!
    attn = einsum("bhqk,bkhd->bqhd", A, v).reshape(B, N, D)
    out = Q + attn @ Wo.T + bo
    ffn = relu(out @ W1.T + b1) @ W2.T + b2
    return out + ffn

Strategy: pure data-parallel over batch: 8 cores x 4 batches, zero
collectives.  On-chip activations are kept in TRANSPOSED layout
([feature, token], feature on partitions) so every matmul contracts over
partitions without any on-chip transposes; host pre-transposes Q/K and
the weights, and re-transposes the output.  Matmuls run in float32r
(full PE rate at moving-dim >= 256, ~tf32 precision).

Softmax over the query axis is computed on scores^T tiles ([k, q],
q on the free axis): ACT exp with fused free-axis accumulation, then the
reciprocal row-sums are folded into v (64x fewer elements than A).
Attention runs per head-PAIR: the two heads of a pair occupy disjoint
row groups (scores, K=64) / col groups (attn-apply, M=64) of the PE
array via tile_position, so their matmuls execute concurrently.
E and v~ are bf16 (attn matmul at full rate; softmax tolerance is wide).
"""

import math
import os
import sys

import numpy as np

sys.path.insert(0, "/opt/trn_rl_repo")

import concourse.bass as bass  # noqa: E402
import concourse.tile as tile  # noqa: E402
from concourse import bacc  # noqa: E402
from concourse import mybir  # noqa: E402
from concourse.bass_utils import run_bass_kernel_spmd  # noqa: E402

F32 = mybir.dt.float32
F32R = mybir.dt.float32r
BF16 = mybir.dt.bfloat16
AF = mybir.ActivationFunctionType
ALU = mybir.AluOpType

B, N, D, H = 32, 512, 512, 8
DH = D // H  # 64
NCORES = 8
BLOC = B // NCORES  # 4 batches per core
SCALE = 1.0 / math.sqrt(512.0)
P = 128
KC = D // P  # 4 contraction chunks
MC = D // P  # 4 output-feature chunks

_CACHE = {}


def _build_program(with_bias):
    nc = bacc.Bacc("TRN2", target_bir_lowering=False, debug=False,
                   num_devices=NCORES)

    # DRAM I/O ------------------------------------------------------------
    qT_d = nc.dram_tensor("qT", [BLOC, D, N], F32R, kind="ExternalInput").ap()
    kT_d = nc.dram_tensor("kT", [BLOC, D, N], F32R, kind="ExternalInput").ap()
    w_d = {}
    for nm in ("wq", "wk", "wv", "wo", "w1", "w2"):
        w_d[nm] = nc.dram_tensor(nm, [D, D], F32R, kind="ExternalInput").ap()
    b_d = {}
    if with_bias:
        for nm in ("bq", "bk", "bv", "bo", "b1", "b2"):
            b_d[nm] = nc.dram_tensor(nm, [D], F32, kind="ExternalInput").ap()
    outT_d = nc.dram_tensor("outT", [BLOC, D, N], F32,
                            kind="ExternalOutput").ap()

    qT_v = qT_d.rearrange("b (o p) t -> b p o t", p=P)
    kT_v = kT_d.rearrange("b (o p) t -> b p o t", p=P)
    outT_v = outT_d.rearrange("b (o p) t -> b p o t", p=P)
    w_v = {k: v.rearrange("(o p) n -> p o n", p=P) for k, v in w_d.items()}
    b_v = {k: v.rearrange("(o p) -> p o", p=P) for k, v in b_d.items()}

    with tile.TileContext(nc) as tc:
        with (
            tc.tile_pool(name="weights", bufs=1) as wpool,
            tc.tile_pool(name="qin", bufs=2) as qin_pool,
            tc.tile_pool(name="kin", bufs=2) as kin_pool,
            tc.tile_pool(name="proj", bufs=2) as proj_pool,
            tc.tile_pool(name="exp", bufs=4) as exp_pool,
            tc.tile_pool(name="rsum", bufs=4) as rsum_pool,
            tc.tile_pool(name="attn", bufs=2) as attn_pool,
            tc.tile_pool(name="ffn", bufs=2) as ffn_pool,
            tc.tile_pool(name="h1p", bufs=2) as h1_pool,
            tc.tile_pool(name="fin", bufs=2) as fin_pool,
            tc.tile_pool(name="psA", bufs=5, space="PSUM") as psA,
            tc.tile_pool(name="psS", bufs=2, space="PSUM") as psS,
            tc.tile_pool(name="psB", bufs=1, space="PSUM") as psB,
        ):
            # ---- resident weights/biases --------------------------------
            w_sb = {}
            for nm in ("wq", "wk", "wv", "wo", "w1", "w2"):
                w_sb[nm] = wpool.tile([P, KC, D], F32R, tag=f"w_{nm}",
                                      name=f"w_{nm}")
            qt0 = qin_pool.tile([P, KC, N], F32R, tag="qt", name="qt0")
            kt0 = kin_pool.tile([P, KC, N], F32R, tag="kt", name="kt0")
            for kc in range(KC):
                nc.sync.dma_start(out=w_sb["wq"][:, kc, :], in_=w_v["wq"][:, kc, :])
                nc.sync.dma_start(out=qt0[:, kc, :], in_=qT_v[0][:, kc, :])
            for kc in range(KC):
                nc.sync.dma_start(out=w_sb["wk"][:, kc, :], in_=w_v["wk"][:, kc, :])
                nc.sync.dma_start(out=kt0[:, kc, :], in_=kT_v[0][:, kc, :])
            nc.sync.dma_start(out=w_sb["wv"][:], in_=w_v["wv"])
            b_sb = {}
            bv_bc = None
            if with_bias:
                for nm in ("bq", "bk", "bo", "b1", "b2"):
                    b_sb[nm] = wpool.tile([P, MC], F32, tag=f"b_{nm}",
                                          name=f"b_{nm}")
                    nc.sync.dma_start(out=b_sb[nm][:], in_=b_v[nm])
                bv_bc = wpool.tile([P, D], F32, tag="bv_bc")
                bv_src = bass.AP(tensor=b_d["bv"].tensor,
                                 offset=b_d["bv"].offset,
                                 ap=[[0, P], *b_d["bv"].ap])
                nc.sync.dma_start(out=bv_bc[:], in_=bv_src)

            def linearT(dst, rhs_src, wname, bias):
                """dst[:, m, :] ([P, MC, N] transposed layout) = W @ rhs + b"""
                for m in range(MC):
                    ps = psA.tile([P, N], F32, tag="psA")
                    for kc in range(KC):
                        nc.tensor.matmul(
                            ps, lhsT=w_sb[wname][:, kc, m * P:(m + 1) * P],
                            rhs=rhs_src[:, kc, :],
                            start=(kc == 0), stop=(kc == KC - 1))
                    if with_bias:
                        nc.vector.tensor_scalar(
                            out=dst[:, m, :], in0=ps,
                            scalar1=b_sb[bias][:, m:m + 1], scalar2=None,
                            op0=ALU.add)
                    else:
                        nc.vector.tensor_copy(out=dst[:, m, :], in_=ps)

            for b in range(BLOC):
                # ---- load inputs (b=0 pre-issued above) ----------------
                if b == 0:
                    qt_b, kt_b = qt0, kt0
                else:
                    qt_b = qin_pool.tile([P, KC, N], F32R, tag="qt")
                    nc.sync.dma_start(out=qt_b[:], in_=qT_v[b])
                    kt_b = kin_pool.tile([P, KC, N], F32R, tag="kt")
                    nc.sync.dma_start(out=kt_b[:], in_=kT_v[b])

                # ---- projections (transposed layout) -------------------
                qh = proj_pool.tile([P, MC, N], BF16, tag="qh")
                linearT(qh, qt_b, "wq", "bq")
                kh = proj_pool.tile([P, MC, N], BF16, tag="kh")
                linearT(kh, kt_b, "wk", "bk")

                # v in natural layout [tok, d']: 4 token chunks
                v_b = proj_pool.tile([P, KC, D], BF16, tag="v")
                for tt in range(KC):
                    ps = psA.tile([P, D], F32, tag="psA")
                    for kc in range(KC):
                        nc.tensor.matmul(
                            ps, lhsT=kt_b[:, kc, tt * P:(tt + 1) * P],
                            rhs=w_sb["wv"][:, kc, :],
                            start=(kc == 0), stop=(kc == KC - 1))
                    if with_bias:
                        nc.vector.tensor_tensor(
                            out=v_b[:, tt, :], in0=ps, in1=bv_bc[:],
                            op=ALU.add)
                    else:
                        nc.vector.tensor_copy(out=v_b[:, tt, :], in_=ps)

                if b == 0:
                    # deferred weight loads: DMA overlaps attention of b=0
                    for nm in ("wo", "w1", "w2"):
                        nc.sync.dma_start(out=w_sb[nm][:], in_=w_v[nm])

                # ---- attention, head pairs -----------------------------
                # pair hp = heads (2hp, 2hp+1): rows 0-63 / 64-127 of
                # feature chunk hp.  Scores row-packed (K=64 x2), attn
                # col-packed (M=64 x2) into one [128, N] psum.
                attnT = attn_pool.tile([P, MC, N], F32R, tag="attnT")
                for hp in range(MC):
                    e0 = exp_pool.tile([P, KC, N], BF16, tag="e", name="e0")
                    e1 = exp_pool.tile([P, KC, N], BF16, tag="e", name="e1")
                    racc = rsum_pool.tile([P, KC, 2], F32, tag="racc")
                    for j in range(KC):
                        js = slice(j * P, (j + 1) * P)
                        ps0 = psS.tile([P, N], F32, tag="psS")
                        nc.tensor.matmul(
                            ps0, lhsT=kh[0:DH, hp, js], rhs=qh[0:DH, hp, :],
                            start=True, stop=True)
                        ps1 = psS.tile([P, N], F32, tag="psS")
                        nc.tensor.matmul(
                            ps1, lhsT=kh[DH:P, hp, js], rhs=qh[DH:P, hp, :],
                            start=True, stop=True)
                        nc.scalar.activation(
                            out=e0[:, j, :], in_=ps0, func=AF.Exp,
                            scale=SCALE, accum_out=racc[:, j, 0:1])
                        nc.scalar.activation(
                            out=e1[:, j, :], in_=ps1, func=AF.Exp,
                            scale=SCALE, accum_out=racc[:, j, 1:2])
                    rrec = rsum_pool.tile([P, KC, 2], F32, tag="rrec")
                    nc.vector.reciprocal(out=rrec[:], in_=racc[:])
                    vt0 = rsum_pool.tile([P, KC, DH], BF16, tag="vt",
                                         name="vt0")
                    vt1 = rsum_pool.tile([P, KC, DH], BF16, tag="vt",
                                         name="vt1")
                    nc.vector.tensor_tensor(
                        out=vt0[:], in0=v_b[:, :, 2 * hp * DH:(2 * hp + 1) * DH],
                        in1=rrec[:, :, 0:1].to_broadcast((P, KC, DH)),
                        op=ALU.mult)
                    nc.vector.tensor_tensor(
                        out=vt1[:], in0=v_b[:, :, (2 * hp + 1) * DH:(2 * hp + 2) * DH],
                        in1=rrec[:, :, 1:2].to_broadcast((P, KC, DH)),
                        op=ALU.mult)
                    ps = psB.tile([P, N], F32, tag="psB")
                    for j in range(KC):
                        nc.tensor.matmul(
                            ps[0:DH, :], lhsT=vt0[:, j, :], rhs=e0[:, j, :],
                            start=(j == 0), stop=(j == KC - 1),
                            tile_position=(0, 0))
                        nc.tensor.matmul(
                            ps[DH:P, :], lhsT=vt1[:, j, :], rhs=e1[:, j, :],
                            start=(j == 0), stop=(j == KC - 1),
                            tile_position=(0, DH))
                    nc.vector.tensor_copy(out=attnT[:, hp, :], in_=ps)

                # ---- out = Q + attn @ Wo.T + bo (transposed) -----------
                outT_b = ffn_pool.tile([P, MC, N], F32R, tag="outT")
                for m in range(MC):
                    ps = psA.tile([P, N], F32, tag="psA")
                    for kc in range(KC):
                        nc.tensor.matmul(
                            ps, lhsT=w_sb["wo"][:, kc, m * P:(m + 1) * P],
                            rhs=attnT[:, kc, :],
                            start=(kc == 0), stop=(kc == KC - 1))
                    if with_bias:
                        nc.vector.tensor_scalar(
                            out=outT_b[:, m, :], in0=ps,
                            scalar1=b_sb["bo"][:, m:m + 1], scalar2=None,
                            op0=ALU.add)
                        nc.vector.tensor_tensor(
                            out=outT_b[:, m, :], in0=outT_b[:, m, :],
                            in1=qt_b[:, m, :], op=ALU.add)
                    else:
                        nc.vector.tensor_tensor(
                            out=outT_b[:, m, :], in0=ps,
                            in1=qt_b[:, m, :], op=ALU.add)

                # ---- ffn h1 = relu(W1 out^T + b1) ----------------------
                h1 = h1_pool.tile([P, MC, N], F32R, tag="h1")
                for m in range(MC):
                    ps = psA.tile([P, N], F32, tag="psA")
                    for kc in range(KC):
                        nc.tensor.matmul(
                            ps, lhsT=w_sb["w1"][:, kc, m * P:(m + 1) * P],
                            rhs=outT_b[:, kc, :],
                            start=(kc == 0), stop=(kc == KC - 1))
                    nc.vector.tensor_scalar(
                        out=h1[:, m, :], in0=ps,
                        scalar1=b_sb["b1"][:, m:m + 1] if with_bias else 0.0,
                        scalar2=0.0,
                        op0=ALU.add, op1=ALU.max)

                # ---- final = out + W2 h1 + b2, DMA out -----------------
                for m in range(MC):
                    ps = psA.tile([P, N], F32, tag="psA")
                    for kc in range(KC):
                        nc.tensor.matmul(
                            ps, lhsT=w_sb["w2"][:, kc, m * P:(m + 1) * P],
                            rhs=h1[:, kc, :],
                            start=(kc == 0), stop=(kc == KC - 1))
                    fin = fin_pool.tile([P, N], F32, tag="fin")
                    if with_bias:
                        nc.scalar.activation(
                            out=fin[:], in_=ps, func=AF.Identity,
                            bias=b_sb["b2"][:, m:m + 1], scale=1.0)
                        nc.vector.tensor_tensor(
                            out=fin[:], in0=fin[:], in1=outT_b[:, m, :],
                            op=ALU.add)
                    else:
                        nc.vector.tensor_tensor(
                            out=fin[:], in0=ps, in1=outT_b[:, m, :],
                            op=ALU.add)
                    nc.sync.dma_start(out=outT_v[b][:, m, :], in_=fin[:])

    nc.compile()
    return nc


def kernel(Q, K, Wq, bq, Wk, bk, Wv, bv, Wo, bo, W1, b1, W2, b2):
    Q = np.asarray(Q, dtype=np.float32)
    K = np.asarray(K, dtype=np.float32)

    biases = {nm: np.asarray(v, np.float32) for nm, v in
              (("bq", bq), ("bk", bk), ("bv", bv),
               ("bo", bo), ("b1", b1), ("b2", b2))}
    with_bias = any(np.any(v) for v in biases.values())

    key = ("nc", with_bias)
    if key not in _CACHE:
        _CACHE[key] = _build_program(with_bias)
    nc = _CACHE[key]

    common = {
        "wq": np.ascontiguousarray(np.asarray(Wq, np.float32).T),
        "wk": np.ascontiguousarray(np.asarray(Wk, np.float32).T),
        "wv": np.ascontiguousarray(np.asarray(Wv, np.float32).T),
        "wo": np.ascontiguousarray(np.asarray(Wo, np.float32).T),
        "w1": np.ascontiguousarray(np.asarray(W1, np.float32).T),
        "w2": np.ascontiguousarray(np.asarray(W2, np.float32).T),
    }
    if with_bias:
        common.update(biases)
    in_maps = []
    for c in range(NCORES):
        sl = slice(c * BLOC, (c + 1) * BLOC)
        in_maps.append({
            "qT": np.ascontiguousarray(Q[sl].transpose(0, 2, 1)),
            "kT": np.ascontiguousarray(K[sl].transpose(0, 2, 1)),
            **common,
        })

    trace = bool(int(os.environ.get("KERNEL_TRACE", "0")))
    res = run_bass_kernel_spmd(nc, in_maps, core_ids=list(range(NCORES)),
                               trace=trace)
    if trace and res.exec_time_ns is not None:
        print(f"HW exec time: {res.exec_time_ns} ns")
        if res.instructions_and_trace is not None:
            print("trace:", res.instructions_and_trace[1])

    out = np.empty((B, N, D), np.float32)
    for c in range(NCORES):
        out[c * BLOC:(c + 1) * BLOC] = res.results[c]["outT"].transpose(0, 2, 1)
    return out


# revision 22
# speedup vs baseline: 1.3050x; 1.2527x over previous
"""Trainium2 Bass kernel for nn_MAB (dense transformer block).

Reference computation (B=32, N=512, D=512, H=8, dh=64):
    q = (Q @ Wq.T + bq)  k = (K @ Wk.T + bk)  v = (K @ Wv.T + bv)
    scores = einsum("bqhd,bkhd->bhqk", q, k) / sqrt(512)
    A = softmax(scores, axis=2)            # over the QUERY axis!
    attn = einsum("bhqk,bkhd->bqhd", A, v).reshape(B, N, D)
    out = Q + attn @ Wo.T + bo
    ffn = relu(out @ W1.T + b1) @ W2.T + b2
    return out + ffn

Strategy: pure data-parallel over batch: 8 cores x 4 batches, zero
collectives.  On-chip activations are kept in TRANSPOSED layout
([feature, token], feature on partitions) so every matmul contracts over
partitions without any on-chip transposes; host pre-transposes Q/K and
the weights, and re-transposes the output.  Matmuls run in float32r
(full PE rate at moving-dim >= 256, ~tf32 precision).

Softmax over the query axis is computed on scores^T tiles ([k, q],
q on the free axis): ACT exp with fused free-axis accumulation, then the
reciprocal row-sums are folded into v (64x fewer elements than A).
Attention runs per head-PAIR: the two heads of a pair occupy disjoint
row groups (scores, K=64) / col groups (attn-apply, M=64) of the PE
array via tile_position, so their matmuls execute concurrently.
E and v~ are bf16 (attn matmul at full rate; softmax tolerance is wide).
"""

import math
import os
import sys

import numpy as np

sys.path.insert(0, "/opt/trn_rl_repo")

import concourse.bass as bass  # noqa: E402
import concourse.tile as tile  # noqa: E402
from concourse import bacc  # noqa: E402
from concourse import mybir  # noqa: E402
from concourse.bass_utils import run_bass_kernel_spmd  # noqa: E402

F32 = mybir.dt.float32
F32R = mybir.dt.float32r
BF16 = mybir.dt.bfloat16
AF = mybir.ActivationFunctionType
ALU = mybir.AluOpType

B, N, D, H = 32, 512, 512, 8
DH = D // H  # 64
NCORES = 8
BLOC = B // NCORES  # 4 batches per core
SCALE = 1.0 / math.sqrt(512.0)
P = 128
KC = D // P  # 4 contraction chunks
MC = D // P  # 4 output-feature chunks

_CACHE = {}


def _build_program(with_bias):
    nc = bacc.Bacc("TRN2", target_bir_lowering=False, debug=False,
                   num_devices=NCORES)

    # DRAM I/O ------------------------------------------------------------
    qT_d = nc.dram_tensor("qT", [BLOC, D, N], F32R, kind="ExternalInput").ap()
    kT_d = nc.dram_tensor("kT", [BLOC, D, N], F32R, kind="ExternalInput").ap()
    w_d = {}
    for nm in ("wq", "wk", "wv", "wo", "w1", "w2"):
        w_d[nm] = nc.dram_tensor(nm, [D, D], F32R, kind="ExternalInput").ap()
    b_d = {}
    if with_bias:
        for nm in ("bq", "bk", "bv", "bo", "b1", "b2"):
            b_d[nm] = nc.dram_tensor(nm, [D], F32, kind="ExternalInput").ap()
    outT_d = nc.dram_tensor("outT", [BLOC, D, N], F32,
                            kind="ExternalOutput").ap()

    qT_v = qT_d.rearrange("b (o p) t -> b p o t", p=P)
    kT_v = kT_d.rearrange("b (o p) t -> b p o t", p=P)
    outT_v = outT_d.rearrange("b (o p) t -> b p o t", p=P)
    w_v = {k: v.rearrange("(o p) n -> p o n", p=P) for k, v in w_d.items()}
    b_v = {k: v.rearrange("(o p) -> p o", p=P) for k, v in b_d.items()}

    with tile.TileContext(nc) as tc:
        with (
            tc.tile_pool(name="weights", bufs=1) as wpool,
            tc.tile_pool(name="qin", bufs=3) as qin_pool,
            tc.tile_pool(name="kin", bufs=2) as kin_pool,
            tc.tile_pool(name="proj", bufs=3) as proj_pool,
            tc.tile_pool(name="exp", bufs=4) as exp_pool,
            tc.tile_pool(name="rsum", bufs=4) as rsum_pool,
            tc.tile_pool(name="attn", bufs=2) as attn_pool,
            tc.tile_pool(name="ffn", bufs=2) as ffn_pool,
            tc.tile_pool(name="h1p", bufs=2) as h1_pool,
            tc.tile_pool(name="fin", bufs=2) as fin_pool,
            tc.tile_pool(name="psA", bufs=5, space="PSUM") as psA,
            tc.tile_pool(name="psS", bufs=2, space="PSUM") as psS,
            tc.tile_pool(name="psB", bufs=1, space="PSUM") as psB,
        ):
            # ---- resident weights/biases --------------------------------
            w_sb = {}
            for nm in ("wq", "wk", "wv", "wo", "w1", "w2"):
                w_sb[nm] = wpool.tile([P, KC, D], F32R, tag=f"w_{nm}",
                                      name=f"w_{nm}")
            qt0 = qin_pool.tile([P, KC, N], F32R, tag="qt", name="qt0")
            kt0 = kin_pool.tile([P, KC, N], F32R, tag="kt", name="kt0")
            for kc in range(KC):
                nc.sync.dma_start(out=w_sb["wq"][:, kc, :], in_=w_v["wq"][:, kc, :])
                nc.sync.dma_start(out=qt0[:, kc, :], in_=qT_v[0][:, kc, :])
            for kc in range(KC):
                nc.sync.dma_start(out=w_sb["wk"][:, kc, :], in_=w_v["wk"][:, kc, :])
                nc.sync.dma_start(out=kt0[:, kc, :], in_=kT_v[0][:, kc, :])
            nc.sync.dma_start(out=w_sb["wv"][:], in_=w_v["wv"])
            b_sb = {}
            bv_bc = None
            if with_bias:
                for nm in ("bq", "bk", "bo", "b1", "b2"):
                    b_sb[nm] = wpool.tile([P, MC], F32, tag=f"b_{nm}",
                                          name=f"b_{nm}")
                    nc.sync.dma_start(out=b_sb[nm][:], in_=b_v[nm])
                bv_bc = wpool.tile([P, D], F32, tag="bv_bc")
                bv_src = bass.AP(tensor=b_d["bv"].tensor,
                                 offset=b_d["bv"].offset,
                                 ap=[[0, P], *b_d["bv"].ap])
                nc.sync.dma_start(out=bv_bc[:], in_=bv_src)

            def linearT(dst, rhs_src, wname, bias):
                """dst[:, m, :] ([P, MC, N] transposed layout) = W @ rhs + b"""
                for m in range(MC):
                    ps = psA.tile([P, N], F32, tag="psA")
                    for kc in range(KC):
                        nc.tensor.matmul(
                            ps, lhsT=w_sb[wname][:, kc, m * P:(m + 1) * P],
                            rhs=rhs_src[:, kc, :],
                            start=(kc == 0), stop=(kc == KC - 1))
                    if with_bias:
                        nc.vector.tensor_scalar(
                            out=dst[:, m, :], in0=ps,
                            scalar1=b_sb[bias][:, m:m + 1], scalar2=None,
                            op0=ALU.add)
                    else:
                        nc.vector.tensor_copy(out=dst[:, m, :], in_=ps)

            st = {}  # per-batch state tiles

            def emit_proj(b):
                if b == 0:
                    qt_b, kt_b = qt0, kt0
                else:
                    qt_b = qin_pool.tile([P, KC, N], F32R, tag="qt")
                    nc.sync.dma_start(out=qt_b[:], in_=qT_v[b])
                    kt_b = kin_pool.tile([P, KC, N], F32R, tag="kt")
                    nc.sync.dma_start(out=kt_b[:], in_=kT_v[b])

                qh = proj_pool.tile([P, MC, N], BF16, tag="qh")
                linearT(qh, qt_b, "wq", "bq")
                kh = proj_pool.tile([P, MC, N], BF16, tag="kh")
                linearT(kh, kt_b, "wk", "bk")

                v_b = proj_pool.tile([P, KC, D], BF16, tag="v")
                for tt in range(KC):
                    ps = psA.tile([P, D], F32, tag="psA")
                    for kc in range(KC):
                        nc.tensor.matmul(
                            ps, lhsT=kt_b[:, kc, tt * P:(tt + 1) * P],
                            rhs=w_sb["wv"][:, kc, :],
                            start=(kc == 0), stop=(kc == KC - 1))
                    if with_bias:
                        nc.vector.tensor_tensor(
                            out=v_b[:, tt, :], in0=ps, in1=bv_bc[:],
                            op=ALU.add)
                    else:
                        nc.vector.tensor_copy(out=v_b[:, tt, :], in_=ps)

                if b == 0:
                    # deferred weight loads: DMA overlaps attention of b=0
                    for nm in ("wo", "w1", "w2"):
                        nc.sync.dma_start(out=w_sb[nm][:], in_=w_v[nm])
                st[b] = {"qt": qt_b, "qh": qh, "kh": kh, "v": v_b}

            def emit_attention(b):
                qh, kh, v_b = st[b]["qh"], st[b]["kh"], st[b]["v"]
                # ---- attention, head pairs -----------------------------
                # pair hp = heads (2hp, 2hp+1): rows 0-63 / 64-127 of
                # feature chunk hp.  Scores row-packed (K=64 x2), attn
                # col-packed (M=64 x2) into one [128, N] psum.
                attnT = attn_pool.tile([P, MC, N], F32R, tag="attnT")
                for hp in range(MC):
                    e0 = exp_pool.tile([P, KC, N], BF16, tag="e", name="e0")
                    e1 = exp_pool.tile([P, KC, N], BF16, tag="e", name="e1")
                    racc = rsum_pool.tile([P, KC, 2], F32, tag="racc")
                    for j in range(KC):
                        js = slice(j * P, (j + 1) * P)
                        ps0 = psS.tile([P, N], F32, tag="psS")
                        nc.tensor.matmul(
                            ps0, lhsT=kh[0:DH, hp, js], rhs=qh[0:DH, hp, :],
                            start=True, stop=True)
                        ps1 = psS.tile([P, N], F32, tag="psS")
                        nc.tensor.matmul(
                            ps1, lhsT=kh[DH:P, hp, js], rhs=qh[DH:P, hp, :],
                            start=True, stop=True)
                        nc.scalar.activation(
                            out=e0[:, j, :], in_=ps0, func=AF.Exp,
                            scale=SCALE, accum_out=racc[:, j, 0:1])
                        nc.scalar.activation(
                            out=e1[:, j, :], in_=ps1, func=AF.Exp,
                            scale=SCALE, accum_out=racc[:, j, 1:2])
                    rrec = rsum_pool.tile([P, KC, 2], F32, tag="rrec")
                    nc.vector.reciprocal(out=rrec[:], in_=racc[:])
                    vt0 = rsum_pool.tile([P, KC, DH], BF16, tag="vt",
                                         name="vt0")
                    vt1 = rsum_pool.tile([P, KC, DH], BF16, tag="vt",
                                         name="vt1")
                    nc.vector.tensor_tensor(
                        out=vt0[:], in0=v_b[:, :, 2 * hp * DH:(2 * hp + 1) * DH],
                        in1=rrec[:, :, 0:1].to_broadcast((P, KC, DH)),
                        op=ALU.mult)
                    nc.vector.tensor_tensor(
                        out=vt1[:], in0=v_b[:, :, (2 * hp + 1) * DH:(2 * hp + 2) * DH],
                        in1=rrec[:, :, 1:2].to_broadcast((P, KC, DH)),
                        op=ALU.mult)
                    ps = psB.tile([P, N], F32, tag="psB")
                    for j in range(KC):
                        nc.tensor.matmul(
                            ps[0:DH, :], lhsT=vt0[:, j, :], rhs=e0[:, j, :],
                            start=(j == 0), stop=(j == KC - 1),
                            tile_position=(0, 0))
                        nc.tensor.matmul(
                            ps[DH:P, :], lhsT=vt1[:, j, :], rhs=e1[:, j, :],
                            start=(j == 0), stop=(j == KC - 1),
                            tile_position=(0, DH))
                    nc.vector.tensor_copy(out=attnT[:, hp, :], in_=ps)
                st[b]["attnT"] = attnT

            def emit_ffn(b):
                attnT, qt_b = st[b]["attnT"], st[b]["qt"]
                # ---- out = Q + attn @ Wo.T + bo (transposed) -----------
                outT_b = ffn_pool.tile([P, MC, N], F32R, tag="outT")
                for m in range(MC):
                    ps = psA.tile([P, N], F32, tag="psA")
                    for kc in range(KC):
                        nc.tensor.matmul(
                            ps, lhsT=w_sb["wo"][:, kc, m * P:(m + 1) * P],
                            rhs=attnT[:, kc, :],
                            start=(kc == 0), stop=(kc == KC - 1))
                    if with_bias:
                        nc.vector.tensor_scalar(
                            out=outT_b[:, m, :], in0=ps,
                            scalar1=b_sb["bo"][:, m:m + 1], scalar2=None,
                            op0=ALU.add)
                        nc.vector.tensor_tensor(
                            out=outT_b[:, m, :], in0=outT_b[:, m, :],
                            in1=qt_b[:, m, :], op=ALU.add)
                    else:
                        nc.vector.tensor_tensor(
                            out=outT_b[:, m, :], in0=ps,
                            in1=qt_b[:, m, :], op=ALU.add)

                # ---- ffn h1 = relu(W1 out^T + b1) ----------------------
                h1 = h1_pool.tile([P, MC, N], F32R, tag="h1")
                for m in range(MC):
                    ps = psA.tile([P, N], F32, tag="psA")
                    for kc in range(KC):
                        nc.tensor.matmul(
                            ps, lhsT=w_sb["w1"][:, kc, m * P:(m + 1) * P],
                            rhs=outT_b[:, kc, :],
                            start=(kc == 0), stop=(kc == KC - 1))
                    nc.vector.tensor_scalar(
                        out=h1[:, m, :], in0=ps,
                        scalar1=b_sb["b1"][:, m:m + 1] if with_bias else 0.0,
                        scalar2=0.0,
                        op0=ALU.add, op1=ALU.max)

                # ---- final = out + W2 h1 + b2, DMA out -----------------
                for m in range(MC):
                    ps = psA.tile([P, N], F32, tag="psA")
                    for kc in range(KC):
                        nc.tensor.matmul(
                            ps, lhsT=w_sb["w2"][:, kc, m * P:(m + 1) * P],
                            rhs=h1[:, kc, :],
                            start=(kc == 0), stop=(kc == KC - 1))
                    fin = fin_pool.tile([P, N], F32, tag="fin")
                    if with_bias:
                        nc.scalar.activation(
                            out=fin[:], in_=ps, func=AF.Identity,
                            bias=b_sb["b2"][:, m:m + 1], scale=1.0)
                        nc.vector.tensor_tensor(
                            out=fin[:], in0=fin[:], in1=outT_b[:, m, :],
                            op=ALU.add)
                    else:
                        nc.vector.tensor_tensor(
                            out=fin[:], in0=ps, in1=outT_b[:, m, :],
                            op=ALU.add)
                    nc.sync.dma_start(out=outT_v[b][:, m, :], in_=fin[:])
                del st[b]

            # software-pipelined emission: proj(b) || attention(b-1)
            # || ffn(b-2) -- lets the scheduler statically interleave
            # PE-heavy projection/FFN work with the ACT-bound softmax.
            for step in range(BLOC + 2):
                if step < BLOC:
                    emit_proj(step)
                if 1 <= step <= BLOC:
                    emit_attention(step - 1)
                if step >= 2:
                    emit_ffn(step - 2)

    nc.compile()
    return nc


def kernel(Q, K, Wq, bq, Wk, bk, Wv, bv, Wo, bo, W1, b1, W2, b2):
    Q = np.asarray(Q, dtype=np.float32)
    K = np.asarray(K, dtype=np.float32)

    biases = {nm: np.asarray(v, np.float32) for nm, v in
              (("bq", bq), ("bk", bk), ("bv", bv),
               ("bo", bo), ("b1", b1), ("b2", b2))}
    with_bias = any(np.any(v) for v in biases.values())

    key = ("nc", with_bias)
    if key not in _CACHE:
        _CACHE[key] = _build_program(with_bias)
    nc = _CACHE[key]

    common = {
        "wq": np.ascontiguousarray(np.asarray(Wq, np.float32).T),
        "wk": np.ascontiguousarray(np.asarray(Wk, np.float32).T),
        "wv": np.ascontiguousarray(np.asarray(Wv, np.float32).T),
        "wo": np.ascontiguousarray(np.asarray(Wo, np.float32).T),
        "w1": np.ascontiguousarray(np.asarray(W1, np.float32).T),
        "w2": np.ascontiguousarray(np.asarray(W2, np.float32).T),
    }
    if with_bias:
        common.update(biases)
    in_maps = []
    for c in range(NCORES):
        sl = slice(c * BLOC, (c + 1) * BLOC)
        in_maps.append({
            "qT": np.ascontiguousarray(Q[sl].transpose(0, 2, 1)),
            "kT": np.ascontiguousarray(K[sl].transpose(0, 2, 1)),
            **common,
        })

    trace = bool(int(os.environ.get("KERNEL_TRACE", "0")))
    res = run_bass_kernel_spmd(nc, in_maps, core_ids=list(range(NCORES)),
                               trace=trace)
    if trace and res.exec_time_ns is not None:
        print(f"HW exec time: {res.exec_time_ns} ns")
        if res.instructions_and_trace is not None:
            print("trace:", res.instructions_and_trace[1])

    out = np.empty((B, N, D), np.float32)
    for c in range(NCORES):
        out[c * BLOC:(c + 1) * BLOC] = res.results[c]["outT"].transpose(0, 2, 1)
    return out


# revision 23
# speedup vs baseline: 1.3369x; 1.0244x over previous
"""Trainium2 Bass kernel for nn_MAB (dense transformer block).

Reference computation (B=32, N=512, D=512, H=8, dh=64):
    q = (Q @ Wq.T + bq)  k = (K @ Wk.T + bk)  v = (K @ Wv.T + bv)
    scores = einsum("bqhd,bkhd->bhqk", q, k) / sqrt(512)
    A = softmax(scores, axis=2)            # over the QUERY axis!
    attn = einsum("bhqk,bkhd->bqhd", A, v).reshape(B, N, D)
    out = Q + attn @ Wo.T + bo
    ffn = relu(out @ W1.T + b1) @ W2.T + b2
    return out + ffn

Strategy: pure data-parallel over batch: 8 cores x 4 batches, zero
collectives.  On-chip activations are kept in TRANSPOSED layout
([feature, token], feature on partitions) so every matmul contracts over
partitions without any on-chip transposes; host pre-transposes Q/K and
the weights, and re-transposes the output.  Matmuls run in float32r
(full PE rate at moving-dim >= 256, ~tf32 precision).

Softmax over the query axis is computed on scores^T tiles ([k, q],
q on the free axis): ACT exp with fused free-axis accumulation, then the
reciprocal row-sums are folded into v (64x fewer elements than A).
Attention runs per head-PAIR: the two heads of a pair occupy disjoint
row groups (scores, K=64) / col groups (attn-apply, M=64) of the PE
array via tile_position, so their matmuls execute concurrently.
E and v~ are bf16 (attn matmul at full rate; softmax tolerance is wide).
"""

import math
import os
import sys

import numpy as np

sys.path.insert(0, "/opt/trn_rl_repo")

import concourse.bass as bass  # noqa: E402
import concourse.tile as tile  # noqa: E402
from concourse import bacc  # noqa: E402
from concourse import mybir  # noqa: E402
from concourse.bass_utils import run_bass_kernel_spmd  # noqa: E402

F32 = mybir.dt.float32
F32R = mybir.dt.float32r
BF16 = mybir.dt.bfloat16
AF = mybir.ActivationFunctionType
ALU = mybir.AluOpType

B, N, D, H = 32, 512, 512, 8
DH = D // H  # 64
NCORES = 8
BLOC = B // NCORES  # 4 batches per core
SCALE = 1.0 / math.sqrt(512.0)
P = 128
KC = D // P  # 4 contraction chunks
MC = D // P  # 4 output-feature chunks

_CACHE = {}


def _build_program(with_bias):
    nc = bacc.Bacc("TRN2", target_bir_lowering=False, debug=False,
                   num_devices=NCORES)

    # DRAM I/O ------------------------------------------------------------
    qT_d = nc.dram_tensor("qT", [BLOC, D, N], F32R, kind="ExternalInput").ap()
    kT_d = nc.dram_tensor("kT", [BLOC, D, N], F32R, kind="ExternalInput").ap()
    w_d = {}
    for nm in ("wq", "wk", "wv", "wo", "w1", "w2"):
        w_d[nm] = nc.dram_tensor(nm, [D, D], F32R, kind="ExternalInput").ap()
    b_d = {}
    if with_bias:
        for nm in ("bq", "bk", "bv", "bo", "b1", "b2"):
            b_d[nm] = nc.dram_tensor(nm, [D], F32, kind="ExternalInput").ap()
    outT_d = nc.dram_tensor("outT", [BLOC, D, N], F32,
                            kind="ExternalOutput").ap()

    qT_v = qT_d.rearrange("b (o p) t -> b p o t", p=P)
    kT_v = kT_d.rearrange("b (o p) t -> b p o t", p=P)
    outT_v = outT_d.rearrange("b (o p) t -> b p o t", p=P)
    w_v = {k: v.rearrange("(o p) n -> p o n", p=P) for k, v in w_d.items()}
    b_v = {k: v.rearrange("(o p) -> p o", p=P) for k, v in b_d.items()}

    with tile.TileContext(nc) as tc:
        with (
            tc.tile_pool(name="weights", bufs=1) as wpool,
            tc.tile_pool(name="qin", bufs=3) as qin_pool,
            tc.tile_pool(name="kin", bufs=2) as kin_pool,
            tc.tile_pool(name="proj", bufs=3) as proj_pool,
            tc.tile_pool(name="exp", bufs=4) as exp_pool,
            tc.tile_pool(name="rsum", bufs=4) as rsum_pool,
            tc.tile_pool(name="attn", bufs=2) as attn_pool,
            tc.tile_pool(name="ffn", bufs=2) as ffn_pool,
            tc.tile_pool(name="h1p", bufs=2) as h1_pool,
            tc.tile_pool(name="fin", bufs=2) as fin_pool,
            tc.tile_pool(name="psA", bufs=5, space="PSUM") as psA,
            tc.tile_pool(name="psS", bufs=2, space="PSUM") as psS,
            tc.tile_pool(name="psB", bufs=1, space="PSUM") as psB,
        ):
            # ---- resident weights/biases --------------------------------
            w_sb = {}
            for nm in ("wq", "wk", "wv", "wo", "w1", "w2"):
                w_sb[nm] = wpool.tile([P, KC, D], F32R, tag=f"w_{nm}",
                                      name=f"w_{nm}")
            qt0 = qin_pool.tile([P, KC, N], F32R, tag="qt", name="qt0")
            kt0 = kin_pool.tile([P, KC, N], F32R, tag="kt", name="kt0")
            for kc in range(KC):
                nc.sync.dma_start(out=w_sb["wq"][:, kc, :], in_=w_v["wq"][:, kc, :])
                nc.sync.dma_start(out=qt0[:, kc, :], in_=qT_v[0][:, kc, :])
            for kc in range(KC):
                nc.sync.dma_start(out=w_sb["wk"][:, kc, :], in_=w_v["wk"][:, kc, :])
                nc.sync.dma_start(out=kt0[:, kc, :], in_=kT_v[0][:, kc, :])
            nc.sync.dma_start(out=w_sb["wv"][:], in_=w_v["wv"])
            b_sb = {}
            bv_bc = None
            if with_bias:
                for nm in ("bq", "bk", "bo", "b1", "b2"):
                    b_sb[nm] = wpool.tile([P, MC], F32, tag=f"b_{nm}",
                                          name=f"b_{nm}")
                    nc.sync.dma_start(out=b_sb[nm][:], in_=b_v[nm])
                bv_bc = wpool.tile([P, D], F32, tag="bv_bc")
                bv_src = bass.AP(tensor=b_d["bv"].tensor,
                                 offset=b_d["bv"].offset,
                                 ap=[[0, P], *b_d["bv"].ap])
                nc.sync.dma_start(out=bv_bc[:], in_=bv_src)

            def linearT(dst, rhs_src, wname, bias):
                """dst[:, m, :] ([P, MC, N] transposed layout) = W @ rhs + b"""
                for m in range(MC):
                    ps = psA.tile([P, N], F32, tag="psA")
                    for kc in range(KC):
                        nc.tensor.matmul(
                            ps, lhsT=w_sb[wname][:, kc, m * P:(m + 1) * P],
                            rhs=rhs_src[:, kc, :],
                            start=(kc == 0), stop=(kc == KC - 1))
                    if with_bias:
                        nc.vector.tensor_scalar(
                            out=dst[:, m, :], in0=ps,
                            scalar1=b_sb[bias][:, m:m + 1], scalar2=None,
                            op0=ALU.add)
                    else:
                        nc.vector.tensor_copy(out=dst[:, m, :], in_=ps)

            st = {}  # per-batch state tiles

            def emit_proj(b):
                if b == 0:
                    qt_b, kt_b = qt0, kt0
                else:
                    qt_b = qin_pool.tile([P, KC, N], F32R, tag="qt")
                    nc.sync.dma_start(out=qt_b[:], in_=qT_v[b])
                    kt_b = kin_pool.tile([P, KC, N], F32R, tag="kt")
                    nc.sync.dma_start(out=kt_b[:], in_=kT_v[b])

                qh = proj_pool.tile([P, MC, N], BF16, tag="qh")
                linearT(qh, qt_b, "wq", "bq")
                kh = proj_pool.tile([P, MC, N], BF16, tag="kh")
                linearT(kh, kt_b, "wk", "bk")

                v_b = proj_pool.tile([P, KC, D], BF16, tag="v")
                for tt in range(KC):
                    ps = psA.tile([P, D], F32, tag="psA")
                    for kc in range(KC):
                        nc.tensor.matmul(
                            ps, lhsT=kt_b[:, kc, tt * P:(tt + 1) * P],
                            rhs=w_sb["wv"][:, kc, :],
                            start=(kc == 0), stop=(kc == KC - 1))
                    if with_bias:
                        nc.vector.tensor_tensor(
                            out=v_b[:, tt, :], in0=ps, in1=bv_bc[:],
                            op=ALU.add)
                    else:
                        nc.vector.tensor_copy(out=v_b[:, tt, :], in_=ps)

                if b == 0:
                    # deferred weight loads: DMA overlaps attention of b=0
                    for nm in ("wo", "w1", "w2"):
                        nc.sync.dma_start(out=w_sb[nm][:], in_=w_v[nm])
                st[b] = {"qt": qt_b, "qh": qh, "kh": kh, "v": v_b}

            def emit_attention(b):
                qh, kh, v_b = st[b]["qh"], st[b]["kh"], st[b]["v"]
                # ---- attention, head pairs -----------------------------
                # pair hp = heads (2hp, 2hp+1): rows 0-63 / 64-127 of
                # feature chunk hp.  Scores row-packed (K=64 x2), attn
                # col-packed (M=64 x2) into one [128, N] psum.
                attnT = attn_pool.tile([P, MC, N], F32R, tag="attnT")
                for hp in range(MC):
                    e0 = exp_pool.tile([P, KC, N], BF16, tag="e", name="e0")
                    e1 = exp_pool.tile([P, KC, N], BF16, tag="e", name="e1")
                    racc = rsum_pool.tile([P, KC, 2], F32, tag="racc")
                    for j in range(KC):
                        js = slice(j * P, (j + 1) * P)
                        ps0 = psS.tile([P, N], F32, tag="psS")
                        nc.tensor.matmul(
                            ps0, lhsT=kh[0:DH, hp, js], rhs=qh[0:DH, hp, :],
                            start=True, stop=True)
                        ps1 = psS.tile([P, N], F32, tag="psS")
                        nc.tensor.matmul(
                            ps1, lhsT=kh[DH:P, hp, js], rhs=qh[DH:P, hp, :],
                            start=True, stop=True)
                        nc.scalar.activation(
                            out=e0[:, j, :], in_=ps0, func=AF.Exp,
                            scale=SCALE, accum_out=racc[:, j, 0:1])
                        nc.scalar.activation(
                            out=e1[:, j, :], in_=ps1, func=AF.Exp,
                            scale=SCALE, accum_out=racc[:, j, 1:2])
                    rrec = rsum_pool.tile([P, KC, 2], F32, tag="rrec")
                    nc.vector.reciprocal(out=rrec[:], in_=racc[:])
                    vt0 = rsum_pool.tile([P, KC, DH], BF16, tag="vt",
                                         name="vt0")
                    vt1 = rsum_pool.tile([P, KC, DH], BF16, tag="vt",
                                         name="vt1")
                    nc.vector.tensor_tensor(
                        out=vt0[:], in0=v_b[:, :, 2 * hp * DH:(2 * hp + 1) * DH],
                        in1=rrec[:, :, 0:1].to_broadcast((P, KC, DH)),
                        op=ALU.mult)
                    nc.vector.tensor_tensor(
                        out=vt1[:], in0=v_b[:, :, (2 * hp + 1) * DH:(2 * hp + 2) * DH],
                        in1=rrec[:, :, 1:2].to_broadcast((P, KC, DH)),
                        op=ALU.mult)
                    ps = psB.tile([P, N], F32, tag="psB")
                    for j in range(KC):
                        nc.tensor.matmul(
                            ps[0:DH, :], lhsT=vt0[:, j, :], rhs=e0[:, j, :],
                            start=(j == 0), stop=(j == KC - 1),
                            tile_position=(0, 0))
                        nc.tensor.matmul(
                            ps[DH:P, :], lhsT=vt1[:, j, :], rhs=e1[:, j, :],
                            start=(j == 0), stop=(j == KC - 1),
                            tile_position=(0, DH))
                    nc.vector.tensor_copy(out=attnT[:, hp, :], in_=ps)
                st[b]["attnT"] = attnT

            def emit_ffn(b):
                attnT, qt_b = st[b]["attnT"], st[b]["qt"]
                # ---- out = Q + attn @ Wo.T + bo (transposed) -----------
                outT_b = ffn_pool.tile([P, MC, N], F32R, tag="outT")
                for m in range(MC):
                    ps = psA.tile([P, N], F32, tag="psA")
                    for kc in range(KC):
                        nc.tensor.matmul(
                            ps, lhsT=w_sb["wo"][:, kc, m * P:(m + 1) * P],
                            rhs=attnT[:, kc, :],
                            start=(kc == 0), stop=(kc == KC - 1))
                    if with_bias:
                        nc.vector.tensor_scalar(
                            out=outT_b[:, m, :], in0=ps,
                            scalar1=b_sb["bo"][:, m:m + 1], scalar2=None,
                            op0=ALU.add)
                        nc.vector.tensor_tensor(
                            out=outT_b[:, m, :], in0=outT_b[:, m, :],
                            in1=qt_b[:, m, :], op=ALU.add)
                    else:
                        nc.vector.tensor_tensor(
                            out=outT_b[:, m, :], in0=ps,
                            in1=qt_b[:, m, :], op=ALU.add)

                # ---- ffn h1 = relu(W1 out^T + b1) ----------------------
                h1 = h1_pool.tile([P, MC, N], F32R, tag="h1")
                for m in range(MC):
                    ps = psA.tile([P, N], F32, tag="psA")
                    for kc in range(KC):
                        nc.tensor.matmul(
                            ps, lhsT=w_sb["w1"][:, kc, m * P:(m + 1) * P],
                            rhs=outT_b[:, kc, :],
                            start=(kc == 0), stop=(kc == KC - 1))
                    nc.vector.tensor_scalar(
                        out=h1[:, m, :], in0=ps,
                        scalar1=b_sb["b1"][:, m:m + 1] if with_bias else 0.0,
                        scalar2=0.0,
                        op0=ALU.add, op1=ALU.max)

                # ---- final = out + W2 h1 + b2, DMA out -----------------
                for m in range(MC):
                    ps = psA.tile([P, N], F32, tag="psA")
                    for kc in range(KC):
                        nc.tensor.matmul(
                            ps, lhsT=w_sb["w2"][:, kc, m * P:(m + 1) * P],
                            rhs=h1[:, kc, :],
                            start=(kc == 0), stop=(kc == KC - 1))
                    fin = fin_pool.tile([P, N], F32, tag="fin")
                    if with_bias:
                        nc.scalar.activation(
                            out=fin[:], in_=ps, func=AF.Identity,
                            bias=b_sb["b2"][:, m:m + 1], scale=1.0)
                        nc.vector.tensor_tensor(
                            out=fin[:], in0=fin[:], in1=outT_b[:, m, :],
                            op=ALU.add)
                    else:
                        nc.vector.tensor_tensor(
                            out=fin[:], in0=ps, in1=outT_b[:, m, :],
                            op=ALU.add)
                    nc.sync.dma_start(out=outT_v[b][:, m, :], in_=fin[:])
                del st[b]

            # software-pipelined emission: proj(b) || attention(b-1)
            # || ffn(b-2) -- lets the scheduler statically interleave
            # PE-heavy projection/FFN work with the ACT-bound softmax.
            for step in range(BLOC + 2):
                if 1 <= step <= BLOC:
                    emit_attention(step - 1)
                if step < BLOC:
                    emit_proj(step)
                if step >= 2:
                    emit_ffn(step - 2)

    nc.compile()
    return nc


def kernel(Q, K, Wq, bq, Wk, bk, Wv, bv, Wo, bo, W1, b1, W2, b2):
    Q = np.asarray(Q, dtype=np.float32)
    K = np.asarray(K, dtype=np.float32)

    biases = {nm: np.asarray(v, np.float32) for nm, v in
              (("bq", bq), ("bk", bk), ("bv", bv),
               ("bo", bo), ("b1", b1), ("b2", b2))}
    with_bias = any(np.any(v) for v in biases.values())

    key = ("nc", with_bias)
    if key not in _CACHE:
        _CACHE[key] = _build_program(with_bias)
    nc = _CACHE[key]

    common = {
        "wq": np.ascontiguousarray(np.asarray(Wq, np.float32).T),
        "wk": np.ascontiguousarray(np.asarray(Wk, np.float32).T),
        "wv": np.ascontiguousarray(np.asarray(Wv, np.float32).T),
        "wo": np.ascontiguousarray(np.asarray(Wo, np.float32).T),
        "w1": np.ascontiguousarray(np.asarray(W1, np.float32).T),
        "w2": np.ascontiguousarray(np.asarray(W2, np.float32).T),
    }
    if with_bias:
        common.update(biases)
    in_maps = []
    for c in range(NCORES):
        sl = slice(c * BLOC, (c + 1) * BLOC)
        in_maps.append({
            "qT": np.ascontiguousarray(Q[sl].transpose(0, 2, 1)),
            "kT": np.ascontiguousarray(K[sl].transpose(0, 2, 1)),
            **common,
        })

    trace = bool(int(os.environ.get("KERNEL_TRACE", "0")))
    res = run_bass_kernel_spmd(nc, in_maps, core_ids=list(range(NCORES)),
                               trace=trace)
    if trace and res.exec_time_ns is not None:
        print(f"HW exec time: {res.exec_time_ns} ns")
        if res.instructions_and_trace is not None:
            print("trace:", res.instructions_and_trace[1])

    out = np.empty((B, N, D), np.float32)
    for c in range(NCORES):
        out[c * BLOC:(c + 1) * BLOC] = res.results[c]["outT"].transpose(0, 2, 1)
    return out


# revision 29
# speedup vs baseline: 1.4115x; 1.0558x over previous
"""Trainium2 Bass kernel for nn_MAB (dense transformer block).

Reference computation (B=32, N=512, D=512, H=8, dh=64):
    q = (Q @ Wq.T + bq)  k = (K @ Wk.T + bk)  v = (K @ Wv.T + bv)
    scores = einsum("bqhd,bkhd->bhqk", q, k) / sqrt(512)
    A = softmax(scores, axis=2)            # over the QUERY axis!
    attn = einsum("bhqk,bkhd->bqhd", A, v).reshape(B, N, D)
    out = Q + attn @ Wo.T + bo
    ffn = relu(out @ W1.T + b1) @ W2.T + b2
    return out + ffn

Strategy: pure data-parallel over batch: 8 cores x 4 batches, zero
collectives.  On-chip activations are kept in TRANSPOSED layout
([feature, token], feature on partitions) so every matmul contracts over
partitions without any on-chip transposes; host pre-transposes Q/K and
the weights, and re-transposes the output.  Matmuls run in float32r
(full PE rate at moving-dim >= 256, ~tf32 precision).

Softmax over the query axis is computed on scores^T tiles ([k, q],
q on the free axis): ACT exp with fused free-axis accumulation, then the
reciprocal row-sums are folded into v (64x fewer elements than A).
Attention runs per head-PAIR: the two heads of a pair occupy disjoint
row groups (scores, K=64) / col groups (attn-apply, M=64) of the PE
array via tile_position, so their matmuls execute concurrently.
E and v~ are bf16 (attn matmul at full rate; softmax tolerance is wide).
"""

import math
import os
import sys

import numpy as np

sys.path.insert(0, "/opt/trn_rl_repo")

import concourse.bass as bass  # noqa: E402
import concourse.tile as tile  # noqa: E402
from concourse import bacc  # noqa: E402
from concourse import mybir  # noqa: E402
from concourse.bass_utils import run_bass_kernel_spmd  # noqa: E402

F32 = mybir.dt.float32
F32R = mybir.dt.float32r
BF16 = mybir.dt.bfloat16
AF = mybir.ActivationFunctionType
ALU = mybir.AluOpType

B, N, D, H = 32, 512, 512, 8
DH = D // H  # 64
NCORES = 8
BLOC = B // NCORES  # 4 batches per core
SCALE = 1.0 / math.sqrt(512.0)
P = 128
KC = D // P  # 4 contraction chunks
MC = D // P  # 4 output-feature chunks

_CACHE = {}


def _build_program(with_bias):
    nc = bacc.Bacc("TRN2", target_bir_lowering=False, debug=False,
                   num_devices=NCORES)

    # DRAM I/O ------------------------------------------------------------
    qT_d = nc.dram_tensor("qT", [BLOC, D, N], F32R, kind="ExternalInput").ap()
    kT_d = nc.dram_tensor("kT", [BLOC, D, N], F32R, kind="ExternalInput").ap()
    w_d = {}
    for nm in ("wq", "wk", "wv", "wo", "w1", "w2"):
        w_d[nm] = nc.dram_tensor(nm, [D, D], F32R, kind="ExternalInput").ap()
    b_d = {}
    if with_bias:
        for nm in ("bq", "bk", "bv", "bo", "b1", "b2"):
            b_d[nm] = nc.dram_tensor(nm, [D], F32, kind="ExternalInput").ap()
    outT_d = nc.dram_tensor("outT", [BLOC, D, N], F32,
                            kind="ExternalOutput").ap()

    qT_v = qT_d.rearrange("b (o p) t -> b p o t", p=P)
    kT_v = kT_d.rearrange("b (o p) t -> b p o t", p=P)
    outT_v = outT_d.rearrange("b (o p) t -> b p o t", p=P)
    w_v = {k: v.rearrange("(o p) n -> p o n", p=P) for k, v in w_d.items()}
    b_v = {k: v.rearrange("(o p) -> p o", p=P) for k, v in b_d.items()}

    with tile.TileContext(nc) as tc:
        with (
            tc.tile_pool(name="weights", bufs=1) as wpool,
            tc.tile_pool(name="qin", bufs=3) as qin_pool,
            tc.tile_pool(name="kin", bufs=2) as kin_pool,
            tc.tile_pool(name="proj", bufs=3) as proj_pool,
            tc.tile_pool(name="exp", bufs=4) as exp_pool,
            tc.tile_pool(name="rsum", bufs=4) as rsum_pool,
            tc.tile_pool(name="attn", bufs=2) as attn_pool,
            tc.tile_pool(name="ffn", bufs=2) as ffn_pool,
            tc.tile_pool(name="h1p", bufs=2) as h1_pool,
            tc.tile_pool(name="fin", bufs=4) as fin_pool,
            tc.tile_pool(name="psA", bufs=3, space="PSUM") as psA,
            tc.tile_pool(name="psS", bufs=4, space="PSUM") as psS,
            tc.tile_pool(name="psB", bufs=1, space="PSUM") as psB,
        ):
            # ---- resident weights/biases --------------------------------
            w_sb = {}
            for nm in ("wq", "wk", "wv", "wo", "w1", "w2"):
                w_sb[nm] = wpool.tile([P, KC, D], F32R, tag=f"w_{nm}",
                                      name=f"w_{nm}")
            qt0 = qin_pool.tile([P, KC, N], F32R, tag="qt", name="qt0")
            kt0 = kin_pool.tile([P, KC, N], F32R, tag="kt", name="kt0")
            for kc in range(KC):
                nc.sync.dma_start(out=w_sb["wq"][:, kc, :], in_=w_v["wq"][:, kc, :])
                nc.sync.dma_start(out=qt0[:, kc, :], in_=qT_v[0][:, kc, :])
            for kc in range(KC):
                nc.sync.dma_start(out=w_sb["wk"][:, kc, :], in_=w_v["wk"][:, kc, :])
                nc.sync.dma_start(out=kt0[:, kc, :], in_=kT_v[0][:, kc, :])
            nc.sync.dma_start(out=w_sb["wv"][:], in_=w_v["wv"])
            b_sb = {}
            bv_bc = None
            if with_bias:
                for nm in ("bq", "bk", "bo", "b1", "b2"):
                    b_sb[nm] = wpool.tile([P, MC], F32, tag=f"b_{nm}",
                                          name=f"b_{nm}")
                    nc.sync.dma_start(out=b_sb[nm][:], in_=b_v[nm])
                bv_bc = wpool.tile([P, D], F32, tag="bv_bc")
                bv_src = bass.AP(tensor=b_d["bv"].tensor,
                                 offset=b_d["bv"].offset,
                                 ap=[[0, P], *b_d["bv"].ap])
                nc.sync.dma_start(out=bv_bc[:], in_=bv_src)

            def linearT(dst, rhs_src, wname, bias):
                """dst[:, m, :] ([P, MC, N] transposed layout) = W @ rhs + b"""
                for m in range(MC):
                    ps = psA.tile([P, N], F32, tag="psA")
                    for kc in range(KC):
                        nc.tensor.matmul(
                            ps, lhsT=w_sb[wname][:, kc, m * P:(m + 1) * P],
                            rhs=rhs_src[:, kc, :],
                            start=(kc == 0), stop=(kc == KC - 1))
                    if with_bias:
                        nc.vector.tensor_scalar(
                            out=dst[:, m, :], in0=ps,
                            scalar1=b_sb[bias][:, m:m + 1], scalar2=None,
                            op0=ALU.add)
                    else:
                        nc.vector.tensor_copy(out=dst[:, m, :], in_=ps)

            st = {}  # per-batch state tiles

            def emit_proj(b):
                if b == 0:
                    qt_b, kt_b = qt0, kt0
                else:
                    qt_b = qin_pool.tile([P, KC, N], F32R, tag="qt")
                    nc.sync.dma_start(out=qt_b[:], in_=qT_v[b])
                    kt_b = kin_pool.tile([P, KC, N], F32R, tag="kt")
                    nc.sync.dma_start(out=kt_b[:], in_=kT_v[b])

                qh = proj_pool.tile([P, MC, N], BF16, tag="qh")
                linearT(qh, qt_b, "wq", "bq")
                kh = proj_pool.tile([P, MC, N], BF16, tag="kh")
                linearT(kh, kt_b, "wk", "bk")

                v_b = proj_pool.tile([P, KC, D], BF16, tag="v")
                for tt in range(KC):
                    ps = psA.tile([P, D], F32, tag="psA")
                    for kc in range(KC):
                        nc.tensor.matmul(
                            ps, lhsT=kt_b[:, kc, tt * P:(tt + 1) * P],
                            rhs=w_sb["wv"][:, kc, :],
                            start=(kc == 0), stop=(kc == KC - 1))
                    if with_bias:
                        nc.vector.tensor_tensor(
                            out=v_b[:, tt, :], in0=ps, in1=bv_bc[:],
                            op=ALU.add)
                    else:
                        nc.vector.tensor_copy(out=v_b[:, tt, :], in_=ps)

                if b == 0:
                    # deferred weight loads: DMA overlaps attention of b=0
                    for nm in ("wo", "w1", "w2"):
                        nc.sync.dma_start(out=w_sb[nm][:], in_=w_v[nm])
                st[b] = {"qt": qt_b, "qh": qh, "kh": kh, "v": v_b}

            def emit_attention(b):
                qh, kh, v_b = st[b]["qh"], st[b]["kh"], st[b]["v"]
                # ---- attention, head pairs -----------------------------
                # pair hp = heads (2hp, 2hp+1): rows 0-63 / 64-127 of
                # feature chunk hp.  Scores row-packed (K=64 x2), attn
                # col-packed (M=64 x2) into one [128, N] psum.
                attnT = attn_pool.tile([P, MC, N], F32R, tag="attnT")
                for hp in range(MC):
                    e0 = exp_pool.tile([P, KC, N], BF16, tag="e", name="e0")
                    e1 = exp_pool.tile([P, KC, N], BF16, tag="e", name="e1")
                    racc = rsum_pool.tile([P, KC, 2], F32, tag="racc")
                    rrec = rsum_pool.tile([P, KC, 2], F32, tag="rrec")
                    vt0 = rsum_pool.tile([P, KC, DH], BF16, tag="vt",
                                         name="vt0")
                    vt1 = rsum_pool.tile([P, KC, DH], BF16, tag="vt",
                                         name="vt1")
                    ps = psB.tile([P, N], F32, tag="psB")
                    for j in range(KC):
                        js = slice(j * P, (j + 1) * P)
                        ps0 = psS.tile([P, N], F32, tag="psS")
                        nc.tensor.matmul(
                            ps0, lhsT=kh[0:DH, hp, js], rhs=qh[0:DH, hp, :],
                            start=True, stop=True)
                        ps1 = psS.tile([P, N], F32, tag="psS")
                        nc.tensor.matmul(
                            ps1, lhsT=kh[DH:P, hp, js], rhs=qh[DH:P, hp, :],
                            start=True, stop=True)
                        nc.scalar.activation(
                            out=e0[:, j, :], in_=ps0, func=AF.Exp,
                            scale=SCALE, accum_out=racc[:, j, 0:1])
                        nc.scalar.activation(
                            out=e1[:, j, :], in_=ps1, func=AF.Exp,
                            scale=SCALE, accum_out=racc[:, j, 1:2])
                        nc.vector.reciprocal(out=rrec[:, j, :],
                                             in_=racc[:, j, :])
                        nc.vector.tensor_tensor(
                            out=vt0[:, j, :],
                            in0=v_b[:, j, 2 * hp * DH:(2 * hp + 1) * DH],
                            in1=rrec[:, j, 0:1].to_broadcast((P, DH)),
                            op=ALU.mult)
                        nc.vector.tensor_tensor(
                            out=vt1[:, j, :],
                            in0=v_b[:, j, (2 * hp + 1) * DH:(2 * hp + 2) * DH],
                            in1=rrec[:, j, 1:2].to_broadcast((P, DH)),
                            op=ALU.mult)
                        nc.tensor.matmul(
                            ps[0:DH, :], lhsT=vt0[:, j, :], rhs=e0[:, j, :],
                            start=(j == 0), stop=(j == KC - 1),
                            tile_position=(0, 0))
                        nc.tensor.matmul(
                            ps[DH:P, :], lhsT=vt1[:, j, :], rhs=e1[:, j, :],
                            start=(j == 0), stop=(j == KC - 1),
                            tile_position=(0, DH))
                    nc.vector.tensor_copy(out=attnT[:, hp, :], in_=ps)
                st[b]["attnT"] = attnT

            def emit_ffn(b):
                attnT, qt_b = st[b]["attnT"], st[b]["qt"]
                # ---- out = Q + attn @ Wo.T + bo (transposed) -----------
                outT_b = ffn_pool.tile([P, MC, N], F32R, tag="outT")
                for m in range(MC):
                    ps = psA.tile([P, N], F32, tag="psA")
                    for kc in range(KC):
                        nc.tensor.matmul(
                            ps, lhsT=w_sb["wo"][:, kc, m * P:(m + 1) * P],
                            rhs=attnT[:, kc, :],
                            start=(kc == 0), stop=(kc == KC - 1))
                    if with_bias:
                        nc.vector.tensor_scalar(
                            out=outT_b[:, m, :], in0=ps,
                            scalar1=b_sb["bo"][:, m:m + 1], scalar2=None,
                            op0=ALU.add)
                        nc.vector.tensor_tensor(
                            out=outT_b[:, m, :], in0=outT_b[:, m, :],
                            in1=qt_b[:, m, :], op=ALU.add)
                    else:
                        nc.vector.tensor_tensor(
                            out=outT_b[:, m, :], in0=ps,
                            in1=qt_b[:, m, :], op=ALU.add)

                # ---- ffn h1 = relu(W1 out^T + b1) ----------------------
                h1 = h1_pool.tile([P, MC, N], F32R, tag="h1")
                for m in range(MC):
                    ps = psA.tile([P, N], F32, tag="psA")
                    for kc in range(KC):
                        nc.tensor.matmul(
                            ps, lhsT=w_sb["w1"][:, kc, m * P:(m + 1) * P],
                            rhs=outT_b[:, kc, :],
                            start=(kc == 0), stop=(kc == KC - 1))
                    nc.vector.tensor_scalar(
                        out=h1[:, m, :], in0=ps,
                        scalar1=b_sb["b1"][:, m:m + 1] if with_bias else 0.0,
                        scalar2=0.0,
                        op0=ALU.add, op1=ALU.max)

                # ---- final = out + W2 h1 + b2, DMA out -----------------
                for m in range(MC):
                    ps = psA.tile([P, N], F32, tag="psA")
                    for kc in range(KC):
                        nc.tensor.matmul(
                            ps, lhsT=w_sb["w2"][:, kc, m * P:(m + 1) * P],
                            rhs=h1[:, kc, :],
                            start=(kc == 0), stop=(kc == KC - 1))
                    fin = fin_pool.tile([P, N], F32, tag="fin")
                    if with_bias:
                        nc.scalar.activation(
                            out=fin[:], in_=ps, func=AF.Identity,
                            bias=b_sb["b2"][:, m:m + 1], scale=1.0)
                        nc.vector.tensor_tensor(
                            out=fin[:], in0=fin[:], in1=outT_b[:, m, :],
                            op=ALU.add)
                    else:
                        nc.vector.tensor_tensor(
                            out=fin[:], in0=ps, in1=outT_b[:, m, :],
                            op=ALU.add)
                    nc.sync.dma_start(out=outT_v[b][:, m, :], in_=fin[:])
                del st[b]

            # software-pipelined emission: proj(b) || attention(b-1)
            # || ffn(b-2) -- lets the scheduler statically interleave
            # PE-heavy projection/FFN work with the ACT-bound softmax.
            for step in range(BLOC + 2):
                if 1 <= step <= BLOC:
                    emit_attention(step - 1)
                if step < BLOC:
                    emit_proj(step)
                if step >= 2:
                    emit_ffn(step - 2)

    nc.compile()
    return nc


def kernel(Q, K, Wq, bq, Wk, bk, Wv, bv, Wo, bo, W1, b1, W2, b2):
    Q = np.asarray(Q, dtype=np.float32)
    K = np.asarray(K, dtype=np.float32)

    biases = {nm: np.asarray(v, np.float32) for nm, v in
              (("bq", bq), ("bk", bk), ("bv", bv),
               ("bo", bo), ("b1", b1), ("b2", b2))}
    with_bias = any(np.any(v) for v in biases.values())

    key = ("nc", with_bias)
    if key not in _CACHE:
        _CACHE[key] = _build_program(with_bias)
    nc = _CACHE[key]

    common = {
        "wq": np.ascontiguousarray(np.asarray(Wq, np.float32).T),
        "wk": np.ascontiguousarray(np.asarray(Wk, np.float32).T),
        "wv": np.ascontiguousarray(np.asarray(Wv, np.float32).T),
        "wo": np.ascontiguousarray(np.asarray(Wo, np.float32).T),
        "w1": np.ascontiguousarray(np.asarray(W1, np.float32).T),
        "w2": np.ascontiguousarray(np.asarray(W2, np.float32).T),
    }
    if with_bias:
        common.update(biases)
    in_maps = []
    for c in range(NCORES):
        sl = slice(c * BLOC, (c + 1) * BLOC)
        in_maps.append({
            "qT": np.ascontiguousarray(Q[sl].transpose(0, 2, 1)),
            "kT": np.ascontiguousarray(K[sl].transpose(0, 2, 1)),
            **common,
        })

    trace = bool(int(os.environ.get("KERNEL_TRACE", "0")))
    res = run_bass_kernel_spmd(nc, in_maps, core_ids=list(range(NCORES)),
                               trace=trace)
    if trace and res.exec_time_ns is not None:
        print(f"HW exec time: {res.exec_time_ns} ns")
        if res.instructions_and_trace is not None:
            print("trace:", res.instructions_and_trace[1])

    out = np.empty((B, N, D), np.float32)
    for c in range(NCORES):
        out[c * BLOC:(c + 1) * BLOC] = res.results[c]["outT"].transpose(0, 2, 1)
    return out
